# revision 3
# baseline (speedup 1.0000x reference)
"""Trainium2 Bass kernel for grouped blockwise-quantized w8a8 MoE GEMM.

Problem (full shapes): E=8 experts, T=8192 tokens, K=2048, N=2048,
quant block BS=128 (B=16 K-blocks).  For expert e over its contiguous
token slice x[t,K] (int8 values stored as int32):

    out[t,n] = (sum_b (sum_k x[t,b,k] * w[e,n,b,k]) * is[t,b] * ws[e,n,b]) + bias[e,n]

Sharding: expert-parallel, one expert per NeuronCore (8 cores).  Each core
gets its expert's 1024-token slice and full weight -> embarrassingly
parallel, no collectives.

Per-core algorithm:
  - Pre-scale operands into fp16:  xs = x * is  (per-token-per-block scale),
    wsn = w * ws (per-n-per-block scale), applied in natural layout with
    per-partition scalars (DVE tensor_scalar, int32 -> fp16 cast fused).
  - Transpose both operands to K-on-partition layout with TensorE
    identity-matmuls (contraction must sit on partitions for PE matmul).
  - Main GEMM: for each (t-tile 128, n-tile 512): accumulate 16 K-block
    matmuls into one PSUM bank (fp32), bias folded in via a K=1 matmul
    with a ones row.  Evict PSUM -> bf16 with ScalarE copy, DMA out.

Precision: fp16 operands carry 11-bit mantissas; x,w are exact there and
is, ws round at 2^-11.  fp32 PSUM accumulation keeps the result well
inside the reference's own bf16 output rounding.
"""

import os
import sys
from contextlib import ExitStack
from dataclasses import dataclass

import numpy as np

for _p in ("/opt/trn_rl_repo",):
    if _p not in sys.path and os.path.isdir(_p):
        sys.path.insert(0, _p)

import concourse.bass as bass  # noqa: E402
import concourse.mybir as mybir  # noqa: E402
import concourse.tile as tile  # noqa: E402
from concourse import bacc  # noqa: E402
from concourse.masks import make_identity  # noqa: E402

F16 = mybir.dt.float16
F32 = mybir.dt.float32
BF16 = mybir.dt.bfloat16
I32 = mybir.dt.int32


@dataclass(frozen=True)
class Cfg:
    Tc: int = 1024  # tokens per core (one expert's slice)
    K: int = 2048
    N: int = 2048
    BS: int = 128  # quant block size along K (= partition count)

    @property
    def B(self):  # K-blocks
        return self.K // self.BS

    @property
    def TT(self):  # 128-row t-tiles
        return self.Tc // 128

    @property
    def NCH(self):  # 128-row n-chunks (w natural layout)
        return self.N // 128

    @property
    def NT(self):  # 512-wide n-tiles for the main matmul
        return self.N // 512


FULL = Cfg()


def build_nc(cfg: Cfg = FULL):
    assert cfg.BS == 128 and cfg.Tc % 128 == 0 and cfg.N % 512 == 0
    nc = bacc.Bacc("TRN2", target_bir_lowering=False, debug=False, num_devices=8)

    x_d = nc.dram_tensor("x", [cfg.Tc, cfg.K], I32, kind="ExternalInput")
    w_d = nc.dram_tensor("w", [cfg.N, cfg.K], I32, kind="ExternalInput")
    is_d = nc.dram_tensor("iscale", [cfg.Tc, cfg.B], F32, kind="ExternalInput")
    ws_d = nc.dram_tensor("wscale", [cfg.N, cfg.B], F32, kind="ExternalInput")
    bias_d = nc.dram_tensor("bias", [1, cfg.N], F32, kind="ExternalInput")
    out_d = nc.dram_tensor("out", [cfg.Tc, cfg.N], BF16, kind="ExternalOutput")

    B, TT, NCH, NT, BS = cfg.B, cfg.TT, cfg.NCH, cfg.NT, cfg.BS

    with tile.TileContext(nc) as tc:
        with ExitStack() as ctx:
            const = ctx.enter_context(tc.tile_pool(name="const", bufs=1))
            xi_pool = ctx.enter_context(tc.tile_pool(name="xi", bufs=2))
            wi_pool = ctx.enter_context(tc.tile_pool(name="wi", bufs=3))
            xs_pool = ctx.enter_context(tc.tile_pool(name="xs", bufs=2))
            wsn_pool = ctx.enter_context(tc.tile_pool(name="wsn", bufs=3))
            resid = ctx.enter_context(tc.tile_pool(name="resid", bufs=1))
            tp_psum = ctx.enter_context(tc.tile_pool(name="tpp", bufs=4, space="PSUM"))
            mm_psum = ctx.enter_context(tc.tile_pool(name="mmp", bufs=4, space="PSUM"))
            out_pool = ctx.enter_context(tc.tile_pool(name="outp", bufs=4))

            # --- constants ---
            ident = const.tile([128, 128], F16)
            make_identity(nc, ident[:])
            ones = const.tile([1, 128], F16)
            nc.vector.memset(ones[:], 1.0)
            bias_f32 = const.tile([1, cfg.N], F32)
            nc.sync.dma_start(bias_f32[:], bias_d[:, :])
            bias_f16 = const.tile([1, cfg.N], F16)
            nc.vector.tensor_copy(bias_f16[:], bias_f32[:])
            # all input scales resident: [128, TT, B] / [128, NCH, B]
            is_all = const.tile([128, TT, B], F32)
            nc.sync.dma_start(
                is_all[:], is_d.ap().rearrange("(i p) b -> p i b", p=128)
            )
            ws_all = const.tile([128, NCH, B], F32)
            nc.sync.dma_start(
                ws_all[:], ws_d.ap().rearrange("(j p) b -> p j b", p=128)
            )

            # --- resident transposed+scaled operands ---
            xsT = resid.tile([128, B, cfg.Tc], F16)  # [k-in-block, b, t]
            wsT = [
                resid.tile([128, B, 512], F16, name=f"wsT{nt}", tag=f"wsT{nt}")
                for nt in range(NT)
            ]

            def prep_x_tile(i):
                xi = xi_pool.tile([128, cfg.K], I32)
                nc.sync.dma_start(xi[:], x_d[i * 128 : (i + 1) * 128, :])
                xs = xs_pool.tile([128, cfg.K], F16)
                for b in range(B):
                    nc.vector.tensor_scalar_mul(
                        xs[:, b * BS : (b + 1) * BS],
                        xi[:, b * BS : (b + 1) * BS],
                        is_all[:, i, b : b + 1],
                    )
                for g in range(B // 4):
                    pt = tp_psum.tile([128, 4, 128], F32)
                    for q in range(4):
                        b = g * 4 + q
                        nc.tensor.matmul(
                            pt[:, q, :],
                            lhsT=xs[:, b * BS : (b + 1) * BS],
                            rhs=ident[:],
                            start=True,
                            stop=True,
                        )
                    nc.scalar.copy(
                        xsT[:, g * 4 : (g + 1) * 4, i * 128 : (i + 1) * 128], pt[:]
                    )

            def prep_w_chunk(j):
                nt, jj = divmod(j, 4)
                wi = wi_pool.tile([128, cfg.K], I32)
                nc.sync.dma_start(wi[:], w_d[j * 128 : (j + 1) * 128, :])
                wsn = wsn_pool.tile([128, cfg.K], F16)
                for b in range(B):
                    nc.vector.tensor_scalar_mul(
                        wsn[:, b * BS : (b + 1) * BS],
                        wi[:, b * BS : (b + 1) * BS],
                        ws_all[:, j, b : b + 1],
                    )
                for g in range(B // 4):
                    pt = tp_psum.tile([128, 4, 128], F32)
                    for q in range(4):
                        b = g * 4 + q
                        nc.tensor.matmul(
                            pt[:, q, :],
                            lhsT=wsn[:, b * BS : (b + 1) * BS],
                            rhs=ident[:],
                            start=True,
                            stop=True,
                        )
                    nc.scalar.copy(
                        wsT[nt][:, g * 4 : (g + 1) * 4, jj * 128 : (jj + 1) * 128],
                        pt[:],
                    )

            def main_mm(nt, tt):
                pm = mm_psum.tile([128, 512], F32)
                nc.tensor.matmul(
                    pm[:],
                    lhsT=ones[:],
                    rhs=bias_f16[:, nt * 512 : (nt + 1) * 512],
                    start=True,
                    stop=False,
                )
                for b in range(B):
                    nc.tensor.matmul(
                        pm[:],
                        lhsT=xsT[:, b, tt * 128 : (tt + 1) * 128],
                        rhs=wsT[nt][:, b, :],
                        start=False,
                        stop=(b == B - 1),
                    )
                ot = out_pool.tile([128, 512], BF16)
                nc.scalar.copy(ot[:], pm[:])
                nc.sync.dma_start(
                    out_d[tt * 128 : (tt + 1) * 128, nt * 512 : (nt + 1) * 512], ot[:]
                )

            for i in range(TT):
                prep_x_tile(i)
            # chunks for nt=0 first, then interleave nt main loops with
            # the next nt's w chunks so DMA/DVE run ahead of the PE.
            for j in range(min(4, NCH)):
                prep_w_chunk(j)
            for nt in range(NT):
                for j in range(4 * (nt + 1), min(4 * (nt + 2), NCH)):
                    prep_w_chunk(j)
                for tt in range(TT):
                    main_mm(nt, tt)

    nc.compile()
    return nc


# ----------------------------------------------------------------------------
# host-side entry
# ----------------------------------------------------------------------------

_CACHED = {}


def _get_nc(cfg: Cfg = FULL):
    if cfg not in _CACHED:
        _CACHED[cfg] = build_nc(cfg)
    return _CACHED[cfg]


def make_in_maps(input, weight, token_count, weight_scale, input_scale, bias):
    E = weight.shape[0]
    tc_arr = np.asarray(token_count).astype(np.int64)
    starts = np.concatenate([[0], np.cumsum(tc_arr)])
    in_maps = []
    for e in range(E):
        s, n = int(starts[e]), int(tc_arr[e])
        in_maps.append(
            {
                "x": np.ascontiguousarray(np.asarray(input)[s : s + n]).astype(
                    np.int32, copy=False
                ),
                "w": np.ascontiguousarray(np.asarray(weight)[e]).astype(
                    np.int32, copy=False
                ),
                "iscale": np.ascontiguousarray(
                    np.asarray(input_scale)[s : s + n]
                ).astype(np.float32, copy=False),
                "wscale": np.ascontiguousarray(np.asarray(weight_scale)[e]).astype(
                    np.float32, copy=False
                ),
                "bias": np.ascontiguousarray(np.asarray(bias)[e]).reshape(1, -1).astype(
                    np.float32, copy=False
                ),
            }
        )
    return in_maps


def run_spmd(in_maps, trace=False, cfg: Cfg = FULL):
    from concourse import bass_utils

    nc = _get_nc(cfg)
    return bass_utils.run_bass_kernel_spmd(
        nc, in_maps, core_ids=list(range(len(in_maps))), trace=trace
    )


def _numpy_fallback(input, weight, token_count, weight_scale, input_scale, bias):
    import ml_dtypes

    E = weight.shape[0]
    tc_arr = np.asarray(token_count).astype(np.int64)
    outs = []
    start = 0
    for e in range(E):
        n_tok = int(tc_arr[e])
        Bn = input_scale.shape[1]
        x = np.asarray(input)[start : start + n_tok].astype(np.float32)
        x = x.reshape(n_tok, Bn, -1)
        w = np.asarray(weight)[e].astype(np.float32).reshape(weight.shape[1], Bn, -1)
        partial = np.einsum("tbk,nbk->tbn", x, w)
        out = np.einsum(
            "tbn,tb,nb->tn",
            partial,
            np.asarray(input_scale)[start : start + n_tok],
            np.asarray(weight_scale)[e],
        )
        out = out + np.asarray(bias)[e]
        outs.append(out.astype(ml_dtypes.bfloat16))
        start += n_tok
    return np.concatenate(outs, axis=0)


def kernel(input, weight, token_count, weight_scale, input_scale, bias):
    input = np.asarray(input)
    weight = np.asarray(weight)
    token_count = np.asarray(token_count)
    weight_scale = np.asarray(weight_scale)
    input_scale = np.asarray(input_scale)
    bias = np.asarray(bias)

    E = weight.shape[0]
    if not (
        E == 8
        and np.all(token_count == input.shape[0] // E)
        and input.shape[0] // E == FULL.Tc
        and input.shape[1] == FULL.K
        and weight.shape[1] == FULL.N
    ):
        # irregular routing / shapes: correctness fallback on host
        return _numpy_fallback(
            input, weight, token_count, weight_scale, input_scale, bias
        )

    in_maps = make_in_maps(
        input, weight, token_count, weight_scale, input_scale, bias
    )
    res = run_spmd(in_maps)
    return np.concatenate([r["out"] for r in res.results], axis=0)
